# revision 16
# baseline (speedup 1.0000x reference)
"""Trainium2 Bass kernel for a custom GAT head layer (p-norm aggregation).

Computation (matches the reference):
  z = h @ fc_w.T
  e = leaky_relu(z@a_src [src] + z@a_dst [dst], 0.01)
  alpha = segment_softmax(e, dst)                  (shift-invariant: emax skipped)
  contrib = |alpha * z[src]|^pc                    (+eps inside abs dropped: negligible)
  s = segment_sum(contrib, dst)
  hn = (s + eps)^(1/pc); batchnorm over nodes; elu

Distribution: edges sorted by dst, grouped into node-blocks (<=128 nodes,
<=2048 edges); contiguous runs of blocks go to the 8 cores, so the softmax
denominator and segment sums are core-local. z/s1/s2 tables are computed
1/8th per core and AllGathered. Only other collective is a [128,2] BN-stats
AllReduce. One SPMD program; all per-core variation is in input tensors.
"""

import math

import numpy as np

import concourse.bass as bass
import concourse.bacc as bacc
import concourse.mybir as mybir
import concourse.tile as tile
from concourse.masks import make_identity

NCORES = 8
P = 128
TPB = 16                # edge tiles per block
BE = P * TPB            # edge slots per block (2048)
ZC = 138                # gather-table row floats: [GP(128) | s1 | pad(9)]
F32 = mybir.dt.float32
BF16 = mybir.dt.bfloat16
F16 = mybir.dt.float16
I32 = mybir.dt.int32
AF = mybir.ActivationFunctionType
ALU = mybir.AluOpType
EPS = 1e-6
BN_EPS = 1e-5
NEG_BIG = -1.0e30


def _build_nc(B, T0, N_total, debug_outs=False):
    """One SPMD program. B: blocks/core, T0: phase-0 node tiles/core,
    N_total: real node count (for BN mean divisor)."""
    ZROWS = NCORES * T0 * P
    nc = bacc.Bacc("TRN2", debug=False, target_bir_lowering=False,
                   num_devices=NCORES)

    # ---- external inputs (per-core values, same shapes) ----
    hT_l = nc.dram_tensor("hT_l", [P, T0 * P], F32, kind="ExternalInput")
    fc_wT = nc.dram_tensor("fc_wT", [P, P], F32, kind="ExternalInput")
    w12 = nc.dram_tensor("w12", [P, 2], F32, kind="ExternalInput")
    pc_rep = nc.dram_tensor("pc_rep", [P, P], F32, kind="ExternalInput")
    ipc_col = nc.dram_tensor("ipc_col", [P, 1], F32, kind="ExternalInput")
    gb2 = nc.dram_tensor("gb2", [P, 2], F32, kind="ExternalInput")
    negpad = nc.dram_tensor("negpad", [P, 1], F32, kind="ExternalInput")
    srcg = nc.dram_tensor("srcg", [B, P, TPB], I32, kind="ExternalInput")
    dstmf = nc.dram_tensor("dstmf", [B, P, TPB], F32, kind="ExternalInput")
    dstg = nc.dram_tensor("dstg", [B, P, TPB], I32, kind="ExternalInput")
    dstl = nc.dram_tensor("dstl", [B, P, TPB], I32, kind="ExternalInput")
    pneg = nc.dram_tensor("pneg", [B, P, TPB], F32, kind="ExternalInput")

    out_d = nc.dram_tensor("out", [B * P, P], F32, kind="ExternalOutput")

    # ---- internal DRAM (offset-0 tensors; indirect-DMA sources need that) ----
    zext_l = nc.dram_tensor("zext_l", [T0 * P, ZC], F32)
    z_full = nc.dram_tensor("z_full", [ZROWS, ZC], F32)
    s2_l = nc.dram_tensor("s2_l", [T0 * P, 1], F32)
    s2_full = nc.dram_tensor("s2_full", [ZROWS, 1], F32)
    lnden_d = nc.dram_tensor("lnden_d", [B * P, 1], F32)
    st_in = nc.dram_tensor("st_in", [P, 2], F32)
    st_out = nc.dram_tensor("st_out", [P, 2], F32)
    if debug_outs:
        zf_dbg = nc.dram_tensor("zf_dbg", [ZROWS, ZC], F32,
                                kind="ExternalOutput")
        s2_dbg = nc.dram_tensor("s2_dbg", [ZROWS, 1], F32,
                                kind="ExternalOutput")
        ld_dbg = nc.dram_tensor("ld_dbg", [B * P, 1], F32,
                                kind="ExternalOutput")
        st_dbg = nc.dram_tensor("st_dbg", [P, 2], F32,
                                kind="ExternalOutput")
        gg_dbg = nc.dram_tensor("gg_dbg", [P, TPB * ZC], F32,
                                kind="ExternalOutput")
        s2v_dbg = nc.dram_tensor("s2v_dbg", [P, TPB], F32,
                                 kind="ExternalOutput")
        num_dbg = nc.dram_tensor("num_dbg", [P, TPB], F32,
                                 kind="ExternalOutput")
        io_dbg = nc.dram_tensor("io_dbg", [P, P], F32,
                                kind="ExternalOutput")
        ms_dbg = nc.dram_tensor("ms_dbg", [P, TPB * P], F32,
                                kind="ExternalOutput")

    groups = [list(range(NCORES))]

    with tile.TileContext(nc) as tc:
        with tc.tile_pool(name="const", bufs=1) as cst:
            fcwT_sb = cst.tile([P, P], F32)
            nc.sync.dma_start(fcwT_sb[:], fc_wT[:])
            w12_sb = cst.tile([P, 2], F32)
            nc.sync.dma_start(w12_sb[:], w12[:])
            pcr_sb = cst.tile([P, P], F32)
            nc.sync.dma_start(pcr_sb[:], pc_rep[:])
            ipc_sb = cst.tile([P, 1], F32)
            nc.sync.dma_start(ipc_sb[:], ipc_col[:])
            gb_sb = cst.tile([P, 2], F32)
            nc.sync.dma_start(gb_sb[:], gb2[:])
            npad_sb = cst.tile([P, 1], F32)
            nc.sync.dma_start(npad_sb[:], negpad[:])
            iota_sb = cst.tile([P, P], F32)
            nc.gpsimd.iota(iota_sb[:], pattern=[[1, P]], channel_multiplier=0,
                           allow_small_or_imprecise_dtypes=True)
            ident_sb = cst.tile([P, P], F32)
            make_identity(nc, ident_sb[:])
            # ln(EPS) column for the pad-count stats correction
            lneps_sb = cst.tile([P, 1], F32)
            nc.gpsimd.memset(lneps_sb[:], float(math.log(EPS)))
            zb = cst.tile([P, 1], F32)
            nc.gpsimd.memset(zb[:], 0.0)
            epsb = cst.tile([P, 1], F32)
            nc.gpsimd.memset(epsb[:], EPS)
            bnepsb = cst.tile([P, 1], F32)
            nc.gpsimd.memset(bnepsb[:], BN_EPS)
            # zero-fill lnden_d so indirect reads never see uninit DRAM
            lz0 = cst.tile([P, B], F32)
            nc.gpsimd.memset(lz0[:], 0.0)
            nc.gpsimd.dma_start(
                lnden_d[:].rearrange("(p x) o -> p (x o)", p=P), lz0[:])

            # persistent stores
            s_store = cst.tile([P, B * P], F32)
            hn_store = cst.tile([P, B * P], F32)
            sumh = cst.tile([P, B], F32)
            sumq = cst.tile([P, B], F32)

            # ================= phase 0: z / GP / s1 / s2 tables =============
            with (
                tc.tile_pool(name="p0", bufs=3) as p0,
                tc.tile_pool(name="p0ps", bufs=2, space="PSUM") as p0ps,
            ):
                for i in range(T0):
                    ht = p0.tile([P, P], F32, tag="ht")
                    nc.sync.dma_start(ht[:], hT_l[:, i * P:(i + 1) * P])
                    zps = p0ps.tile([P, P], F32, tag="zps")
                    nc.tensor.matmul(zps[:], ht[:], fcwT_sb[:],
                                     start=True, stop=True)
                    s12 = p0ps.tile([P, 2], F32, tag="s12")
                    nc.tensor.matmul(s12[:], ht[:], w12_sb[:],
                                     start=True, stop=True)
                    az = p0.tile([P, P], F32, tag="az")
                    nc.scalar.activation(az[:], zps[:], AF.Abs, bias=zb[:])
                    lnz = p0.tile([P, P], F32, tag="lnz")
                    nc.scalar.activation(lnz[:], az[:], AF.Ln, bias=zb[:])
                    zx = p0.tile([P, ZC], F32, tag="zx")
                    nc.vector.tensor_tensor(zx[:, 0:P], lnz[:], pcr_sb[:],
                                            op=ALU.mult)
                    nc.vector.tensor_copy(zx[:, P:P + 1], s12[:, 0:1])
                    nc.gpsimd.memset(zx[:, P + 1:ZC], 0.0)
                    s2c = p0.tile([P, 1], F32, tag="s2c")
                    nc.vector.tensor_copy(s2c[:], s12[:, 1:2])
                    nc.sync.dma_start(zext_l[i * P:(i + 1) * P, :], zx[:])
                    nc.sync.dma_start(s2_l[i * P:(i + 1) * P, :], s2c[:])

            # ================= allgather tables =============================
            nc.gpsimd.collective_compute(
                "AllGather", ALU.bypass, replica_groups=groups,
                ins=[zext_l[:]], outs=[z_full[:]])
            nc.gpsimd.collective_compute(
                "AllGather", ALU.bypass, replica_groups=groups,
                ins=[s2_l[:]], outs=[s2_full[:]])

            # ================= edge phase ===================================
            with (
                tc.tile_pool(name="eidx", bufs=3) as eidx,
                tc.tile_pool(name="egg", bufs=2) as egg,
                tc.tile_pool(name="esm", bufs=3) as esm,
                tc.tile_pool(name="ework", bufs=3) as ework,
                tc.tile_pool(name="eps1", bufs=2, space="PSUM") as epsA,
                tc.tile_pool(name="eps2", bufs=2, space="PSUM") as epsB,
            ):
                for b in range(B):
                    srcb = eidx.tile([P, TPB], I32, tag="srcb")
                    nc.sync.dma_start(srcb[:], srcg[b][:])
                    dmb = eidx.tile([P, TPB], F32, tag="dmb")
                    nc.sync.dma_start(dmb[:], dstmf[b][:])
                    dgb = eidx.tile([P, TPB], I32, tag="dgb")
                    nc.sync.dma_start(dgb[:], dstg[b][:])
                    dlb = eidx.tile([P, TPB], I32, tag="dlb")
                    nc.sync.dma_start(dlb[:], dstl[b][:])
                    pnb = eidx.tile([P, TPB], F32, tag="pnb")
                    nc.sync.dma_start(pnb[:], pneg[b][:])

                    gg = egg.tile([P, TPB, ZC], F32, tag="gg")
                    for t in range(TPB):
                        nc.gpsimd.indirect_dma_start(
                            out=gg[:, t, :], out_offset=None, in_=z_full[:],
                            in_offset=bass.IndirectOffsetOnAxis(
                                ap=srcb[:, t:t + 1], axis=0))
                    s2v = esm.tile([P, TPB, 1], F32, tag="s2v")
                    for t in range(TPB):
                        nc.gpsimd.indirect_dma_start(
                            out=s2v[:, t, :], out_offset=None, in_=s2_full[:],
                            in_offset=bass.IndirectOffsetOnAxis(
                                ap=dgb[:, t:t + 1], axis=0))

                    # e = lrelu(s1[src] + s2[dst]) with -1e30 on pad slots
                    ev = esm.tile([P, TPB], F32, tag="ev")
                    nc.vector.tensor_tensor(ev[:], gg[:, :, P], s2v[:, :, 0],
                                            op=ALU.add)
                    nc.vector.tensor_tensor(ev[:], ev[:], pnb[:], op=ALU.add)
                    elr = esm.tile([P, TPB], F32, tag="elr")
                    nc.vector.scalar_tensor_tensor(
                        elr[:], in0=ev[:], scalar=0.01, in1=ev[:],
                        op0=ALU.mult, op1=ALU.max)
                    num = esm.tile([P, TPB], F32, tag="num")
                    nc.scalar.activation(num[:], elr[:], AF.Exp, bias=zb[:])
                    numb = esm.tile([P, TPB], F16, tag="numb")
                    nc.vector.tensor_copy(numb[:], num[:])

                    # selection matrix M[e, t, j] = (dstm[e,t] == j)
                    msel = egg.tile([P, TPB, P], F16, tag="msel")
                    nc.vector.tensor_tensor(
                        msel[:],
                        dmb[:, :, None].broadcast_to([P, TPB, P]),
                        iota_sb[:, None, :].broadcast_to([P, TPB, P]),
                        op=ALU.is_equal)

                    if debug_outs and b == 0:
                        nc.sync.dma_start(
                            gg_dbg[:], gg[:].rearrange("p t c -> p (t c)"))
                        nc.sync.dma_start(s2v_dbg[:], s2v[:, :, 0])
                        nc.sync.dma_start(num_dbg[:], num[:])
                        nc.sync.dma_start(io_dbg[:], iota_sb[:])
                        msf = ework.tile([P, TPB * P], F32, tag="msf")
                        nc.vector.tensor_copy(
                            msf[:], msel[:].rearrange("p t c -> p (t c)"))
                        nc.sync.dma_start(ms_dbg[:], msf[:])
                    denp = epsA.tile([P, 1], F32, tag="denp")
                    for t in range(TPB):
                        nc.tensor.matmul(denp[:], msel[:, t, :],
                                         numb[:, t:t + 1],
                                         start=(t == 0), stop=(t == TPB - 1))
                    dclamp = esm.tile([P, 1], F32, tag="dclamp")
                    nc.vector.tensor_scalar(dclamp[:], denp[:], 1e-30, None,
                                            ALU.max)
                    lnden = esm.tile([P, 1], F32, tag="lnden")
                    nc.scalar.activation(lnden[:], dclamp[:], AF.Ln, bias=zb[:])
                    nc.sync.dma_start(lnden_d[b * P:(b + 1) * P, :], lnden[:])
                    lndv = esm.tile([P, TPB, 1], F32, tag="lndv")
                    for t in range(TPB):
                        nc.gpsimd.indirect_dma_start(
                            out=lndv[:, t, :], out_offset=None, in_=lnden_d[:],
                            in_offset=bass.IndirectOffsetOnAxis(
                                ap=dlb[:, t:t + 1], axis=0))
                    lnnum = esm.tile([P, TPB], F32, tag="lnnum")
                    nc.scalar.activation(lnnum[:], numb[:], AF.Ln, bias=zb[:])
                    lna = esm.tile([P, TPB], F32, tag="lna")
                    nc.vector.tensor_tensor(lna[:], lnnum[:], lndv[:, :, 0],
                                            op=ALU.subtract)

                    sacc = epsB.tile([P, P], F32, tag="sacc")
                    for t in range(TPB):
                        pre = ework.tile([P, P], F32, tag="pre")
                        nc.vector.scalar_tensor_tensor(
                            pre[:], in0=pcr_sb[:], scalar=lna[:, t:t + 1],
                            in1=gg[:, t, 0:P], op0=ALU.mult, op1=ALU.add)
                        ctrb = ework.tile([P, P], F16, tag="ctrb")
                        nc.scalar.activation(ctrb[:], pre[:], AF.Exp, bias=zb[:])
                        nc.tensor.matmul(sacc[:], ctrb[:], msel[:, t, :],
                                         start=(t == 0), stop=(t == TPB - 1))
                    nc.scalar.activation(s_store[:, b * P:(b + 1) * P],
                                         sacc[:], AF.Copy)

            if debug_outs:
                nc.sync.dma_start(zf_dbg[:], z_full[:])
                nc.sync.dma_start(s2_dbg[:], s2_full[:])

            # ================= post: hn, BN stats ===========================
            with (
                tc.tile_pool(name="post", bufs=3) as po,
                tc.tile_pool(name="pops", bufs=2, space="PSUM") as pops,
            ):
                for b in range(B):
                    sb = s_store[:, b * P:(b + 1) * P]
                    lnb = po.tile([P, P], F32, tag="lnb")
                    nc.scalar.activation(lnb[:], sb, AF.Ln, bias=epsb[:])
                    nc.scalar.activation(hn_store[:, b * P:(b + 1) * P],
                                         lnb[:], AF.Exp, bias=zb[:],
                                         scale=ipc_sb[:],
                                         accum_out=sumh[:, b:b + 1])
                    hnq = po.tile([P, P], F32, tag="hnq")
                    nc.scalar.activation(hnq[:],
                                         hn_store[:, b * P:(b + 1) * P],
                                         AF.Square, bias=zb[:],
                                         accum_out=sumq[:, b:b + 1])

                # raw per-core sums, pad-corrected:  sum -= npad * eps^(1/pc)
                epspow = po.tile([P, 1], F32)
                nc.scalar.activation(epspow[:], lneps_sb[:], AF.Exp,
                                     bias=zb[:], scale=ipc_sb[:])
                epsq = po.tile([P, 1], F32)
                nc.scalar.activation(epsq[:], epspow[:], AF.Square, bias=zb[:])
                rs_h = po.tile([P, 1], F32)
                nc.vector.tensor_reduce(rs_h[:], sumh[:], mybir.AxisListType.X,
                                        ALU.add)
                rs_q = po.tile([P, 1], F32)
                nc.vector.tensor_reduce(rs_q[:], sumq[:], mybir.AxisListType.X,
                                        ALU.add)
                stat = po.tile([P, 2], F32)
                nc.vector.scalar_tensor_tensor(
                    stat[:, 0:1], in0=epspow[:], scalar=npad_sb[:],
                    in1=rs_h[:], op0=ALU.mult, op1=ALU.add)
                nc.vector.scalar_tensor_tensor(
                    stat[:, 1:2], in0=epsq[:], scalar=npad_sb[:],
                    in1=rs_q[:], op0=ALU.mult, op1=ALU.add)
                nc.sync.dma_start(st_in[:], stat[:])
                nc.gpsimd.collective_compute(
                    "AllReduce", ALU.add, replica_groups=groups,
                    ins=[st_in[:]], outs=[st_out[:]])
                gstat = po.tile([P, 2], F32)
                nc.sync.dma_start(gstat[:], st_out[:])

                invn = 1.0 / float(N_total)
                mean = po.tile([P, 1], F32)
                nc.vector.tensor_scalar(mean[:], gstat[:, 0:1], invn, None,
                                        ALU.mult)
                exq = po.tile([P, 1], F32)
                nc.vector.tensor_scalar(exq[:], gstat[:, 1:2], invn, None,
                                        ALU.mult)
                nmean = po.tile([P, 1], F32)
                nc.vector.tensor_scalar(nmean[:], mean[:], -1.0, None,
                                        ALU.mult)
                var = po.tile([P, 1], F32)
                nc.vector.scalar_tensor_tensor(
                    var[:], in0=mean[:], scalar=nmean[:], in1=exq[:],
                    op0=ALU.mult, op1=ALU.add)
                # rstd = exp(-0.5 * ln(var + bn_eps))  (stays in the act table)
                lv = po.tile([P, 1], F32)
                nc.scalar.activation(lv[:], var[:], AF.Ln, bias=bnepsb[:])
                rstd = po.tile([P, 1], F32)
                nc.scalar.activation(rstd[:], lv[:], AF.Exp, bias=zb[:], scale=-0.5)
                acol = po.tile([P, 1], F32)
                nc.vector.tensor_tensor(acol[:], gb_sb[:, 0:1], rstd[:],
                                        op=ALU.mult)
                nacol = po.tile([P, 1], F32)
                nc.vector.tensor_scalar(nacol[:], acol[:], -1.0, None,
                                        ALU.mult)
                bcol = po.tile([P, 1], F32)
                nc.vector.scalar_tensor_tensor(
                    bcol[:], in0=mean[:], scalar=nacol[:], in1=gb_sb[:, 1:2],
                    op0=ALU.mult, op1=ALU.add)

                # normalize + elu + transpose + write out
                for b in range(B):
                    v = po.tile([P, P], F32, tag="v")
                    nc.vector.tensor_scalar(v[:],
                                            hn_store[:, b * P:(b + 1) * P],
                                            acol[:], bcol[:],
                                            ALU.mult, ALU.add)
                    em = po.tile([P, P], F32, tag="em")
                    nc.scalar.activation(em[:], v[:], AF.Exp, bias=zb[:])
                    t1 = po.tile([P, P], F32, tag="t1")
                    nc.vector.tensor_scalar(t1[:], em[:], 1.0, 0.0,
                                            ALU.subtract, ALU.min)
                    el = po.tile([P, P], F32, tag="el")
                    nc.vector.scalar_tensor_tensor(
                        el[:], in0=v[:], scalar=0.0, in1=t1[:],
                        op0=ALU.max, op1=ALU.add)
                    tp = pops.tile([P, P], F32, tag="tp")
                    nc.tensor.transpose(tp[:], el[:], ident_sb[:])
                    ot = po.tile([P, P], F32, tag="ot")
                    nc.scalar.activation(ot[:], tp[:], AF.Copy)
                    nc.sync.dma_start(out_d[b * P:(b + 1) * P, :], ot[:])
                if debug_outs:
                    nc.sync.dma_start(ld_dbg[:], lnden_d[:])
                    nc.sync.dma_start(st_dbg[:], st_out[:])

    nc.compile()
    return nc


def _host_prep(h, src, dst, fc_w, attn_w, p, bn_gamma, bn_beta):
    N, DIN = h.shape
    DOUT = fc_w.shape[0]
    assert DIN == P and DOUT == P
    E = src.shape[0]

    # phase-0 split
    T0 = int(math.ceil(N / (NCORES * P)))
    ZROWS = NCORES * T0 * P
    hT = np.ones((P, ZROWS), dtype=np.float32)
    hT[:, :N] = np.ascontiguousarray(h.T)

    # sort edges by dst
    order = np.argsort(dst, kind="stable")
    src_s = np.ascontiguousarray(src[order]).astype(np.int64)
    dst_s = np.ascontiguousarray(dst[order]).astype(np.int64)
    deg = np.bincount(dst_s, minlength=N)
    starts = np.zeros(N + 1, dtype=np.int64)
    np.cumsum(deg, out=starts[1:])
    assert deg.max() <= BE, "single node exceeds block capacity"

    # greedy blocks: <=P nodes, <=BE edges, whole nodes
    blk_lo, blk_hi = [], []          # node ranges
    n = 0
    while n < N:
        m = int(np.searchsorted(starts, starts[n] + BE, side="right")) - 1
        m = min(m, n + P, N)
        m = max(m, n + 1)
        blk_lo.append(n)
        blk_hi.append(m)
        n = m
    nb = len(blk_lo)
    B = int(math.ceil(nb / NCORES))
    blk_lo = np.array(blk_lo + [N] * (NCORES * B - nb), dtype=np.int64)
    blk_hi = np.array(blk_hi + [N] * (NCORES * B - nb), dtype=np.int64)

    # per-core per-block slot arrays
    srcg = np.zeros((NCORES, B, P, TPB), dtype=np.int32)
    dstmf = np.zeros((NCORES, B, P, TPB), dtype=np.float32)
    dstg = np.zeros((NCORES, B, P, TPB), dtype=np.int32)
    dstl = np.zeros((NCORES, B, P, TPB), dtype=np.int32)
    pneg = np.full((NCORES, B, P, TPB), NEG_BIG, dtype=np.float32)
    for c in range(NCORES):
        for b in range(B):
            g = c * B + b
            lo, hi = blk_lo[g], blk_hi[g]
            e0, e1 = starts[lo], starts[hi]
            k = int(e1 - e0)
            dstl[c, b] += b * P  # pad slots still point at a valid row
            if k == 0:
                continue
            off = np.arange(k)
            pp = off % P
            tt = off // P
            srcg[c, b, pp, tt] = src_s[e0:e1]
            dm = (dst_s[e0:e1] - lo)
            dstmf[c, b, pp, tt] = dm.astype(np.float32)
            dstg[c, b, pp, tt] = dst_s[e0:e1]
            dstl[c, b, pp, tt] = (b * P + dm).astype(np.int32)
            pneg[c, b, pp, tt] = 0.0

    # weights / constants
    a_src = attn_w[0, :DOUT].astype(np.float32)
    a_dst = attn_w[0, DOUT:].astype(np.float32)
    fc_wT = np.ascontiguousarray(fc_w.T.astype(np.float32))
    w12 = np.ascontiguousarray(fc_wT @ np.stack([a_src, a_dst], 1))
    pc = np.clip(p.astype(np.float32), 1.0, 100.0)
    pc_rep = np.ascontiguousarray(np.tile(pc[None, :], (P, 1)))
    ipc_col = np.ascontiguousarray((1.0 / pc)[:, None])
    gb2 = np.ascontiguousarray(
        np.stack([bn_gamma.astype(np.float32), bn_beta.astype(np.float32)], 1))

    total_slots = NCORES * B * P
    npad_global = float(total_slots - N)

    in_maps = []
    for c in range(NCORES):
        # per-core pad slots do not matter here: correction uses the global
        # count, applied identically on every core before the AllReduce —
        # sum over cores counts each core's own pads once.
        nslots_c = B * P
        nreal_c = int(np.sum(np.minimum(blk_hi[c * B:(c + 1) * B], N)
                             - np.minimum(blk_lo[c * B:(c + 1) * B], N)))
        npad_c = float(nslots_c - nreal_c)
        in_maps.append({
            "hT_l": np.ascontiguousarray(
                hT[:, c * T0 * P:(c + 1) * T0 * P]),
            "fc_wT": fc_wT,
            "w12": w12.astype(np.float32),
            "pc_rep": pc_rep,
            "ipc_col": ipc_col,
            "gb2": gb2,
            "negpad": np.full((P, 1), -npad_c, dtype=np.float32),
            "srcg": srcg[c],
            "dstmf": dstmf[c],
            "dstg": dstg[c],
            "dstl": dstl[c],
            "pneg": pneg[c],
        })

    meta = dict(B=B, T0=T0, N=N, blk_lo=blk_lo, blk_hi=blk_hi)
    return in_maps, meta


def kernel(h, src, dst, fc_w, attn_w, p, bn_gamma, bn_beta):
    in_maps, meta = _host_prep(h, src, dst, fc_w, attn_w, p,
                               bn_gamma, bn_beta)
    B, T0, N = meta["B"], meta["T0"], meta["N"]
    nc = _build_nc(B, T0, N)

    from concourse.bass_utils import run_bass_kernel_spmd
    res = run_bass_kernel_spmd(nc, in_maps, core_ids=list(range(NCORES)))
    global _last_results, _last_ctx
    _last_results = res
    _last_ctx = (nc, in_maps)
    outs = [r["out"] for r in res.results]

    blk_lo, blk_hi = meta["blk_lo"], meta["blk_hi"]
    full = np.empty((N, P), dtype=np.float32)
    for c in range(NCORES):
        for b in range(B):
            g = c * B + b
            lo, hi = int(blk_lo[g]), int(blk_hi[g])
            cnt = hi - lo
            if cnt > 0:
                full[lo:hi] = outs[c][b * P:b * P + cnt]
    return full


def bench_warm(n=3):
    """Re-execute the last-built program n times (jit cache warm); min wall."""
    import time
    from concourse.bass_utils import run_bass_kernel_spmd
    nc, in_maps = _last_ctx
    best = float("inf")
    for _ in range(n):
        t0 = time.time()
        run_bass_kernel_spmd(nc, in_maps, core_ids=list(range(NCORES)))
        best = min(best, time.time() - t0)
    return best * 1e9


# revision 17
# speedup vs baseline: 1.0213x; 1.0213x over previous
"""Trainium2 Bass kernel for a custom GAT head layer (p-norm aggregation).

Computation (matches the reference):
  z = h @ fc_w.T
  e = leaky_relu(z@a_src [src] + z@a_dst [dst], 0.01)
  alpha = segment_softmax(e, dst)                  (shift-invariant: emax skipped)
  contrib = |alpha * z[src]|^pc                    (+eps inside abs dropped: negligible)
  s = segment_sum(contrib, dst)
  hn = (s + eps)^(1/pc); batchnorm over nodes; elu

Distribution: edges sorted by dst, grouped into node-blocks (<=128 nodes,
<=2048 edges); contiguous runs of blocks go to the 8 cores, so the softmax
denominator and segment sums are core-local. z/s1/s2 tables are computed
1/8th per core and AllGathered. Only other collective is a [128,2] BN-stats
AllReduce. One SPMD program; all per-core variation is in input tensors.
"""

import math

import numpy as np

import concourse.bass as bass
import concourse.bacc as bacc
import concourse.mybir as mybir
import concourse.tile as tile
from concourse.masks import make_identity

NCORES = 8
P = 128
TPB = 16                # edge tiles per block
BE = P * TPB            # edge slots per block (2048)
ZC = 138                # gather-table row floats: [GP(128) | s1 | pad(9)]
F32 = mybir.dt.float32
BF16 = mybir.dt.bfloat16
F16 = mybir.dt.float16
I32 = mybir.dt.int32
AF = mybir.ActivationFunctionType
ALU = mybir.AluOpType
EPS = 1e-6
BN_EPS = 1e-5
NEG_BIG = -1.0e30


def _build_nc(B, T0, N_total, debug_outs=False):
    """One SPMD program. B: blocks/core, T0: phase-0 node tiles/core,
    N_total: real node count (for BN mean divisor)."""
    ZROWS = NCORES * T0 * P
    nc = bacc.Bacc("TRN2", debug=False, target_bir_lowering=False,
                   num_devices=NCORES)

    # ---- external inputs (per-core values, same shapes) ----
    hT_l = nc.dram_tensor("hT_l", [P, T0 * P], F32, kind="ExternalInput")
    fc_wT = nc.dram_tensor("fc_wT", [P, P], F32, kind="ExternalInput")
    w12 = nc.dram_tensor("w12", [P, 2], F32, kind="ExternalInput")
    pc_rep = nc.dram_tensor("pc_rep", [P, P], F32, kind="ExternalInput")
    ipc_col = nc.dram_tensor("ipc_col", [P, 1], F32, kind="ExternalInput")
    gb2 = nc.dram_tensor("gb2", [P, 2], F32, kind="ExternalInput")
    negpad = nc.dram_tensor("negpad", [P, 1], F32, kind="ExternalInput")
    srcg = nc.dram_tensor("srcg", [B, P, TPB], I32, kind="ExternalInput")
    dstmf = nc.dram_tensor("dstmf", [B, P, TPB], F32, kind="ExternalInput")
    dstg = nc.dram_tensor("dstg", [B, P, TPB], I32, kind="ExternalInput")
    dstl = nc.dram_tensor("dstl", [B, P, TPB], I32, kind="ExternalInput")
    pneg = nc.dram_tensor("pneg", [B, P, TPB], F32, kind="ExternalInput")

    out_d = nc.dram_tensor("out", [B * P, P], F32, kind="ExternalOutput")

    # ---- internal DRAM (offset-0 tensors; indirect-DMA sources need that) ----
    zext_l = nc.dram_tensor("zext_l", [T0 * P, ZC], F32)
    z_full = nc.dram_tensor("z_full", [ZROWS, ZC], F32)
    s2_l = nc.dram_tensor("s2_l", [T0 * P, 1], F32)
    s2_full = nc.dram_tensor("s2_full", [ZROWS, 1], F32)
    lnden_d = nc.dram_tensor("lnden_d", [B * P, 1], F32)
    st_in = nc.dram_tensor("st_in", [P, 2], F32)
    st_out = nc.dram_tensor("st_out", [P, 2], F32)
    if debug_outs:
        zf_dbg = nc.dram_tensor("zf_dbg", [ZROWS, ZC], F32,
                                kind="ExternalOutput")
        s2_dbg = nc.dram_tensor("s2_dbg", [ZROWS, 1], F32,
                                kind="ExternalOutput")
        ld_dbg = nc.dram_tensor("ld_dbg", [B * P, 1], F32,
                                kind="ExternalOutput")
        st_dbg = nc.dram_tensor("st_dbg", [P, 2], F32,
                                kind="ExternalOutput")
        gg_dbg = nc.dram_tensor("gg_dbg", [P, TPB * ZC], F32,
                                kind="ExternalOutput")
        s2v_dbg = nc.dram_tensor("s2v_dbg", [P, TPB], F32,
                                 kind="ExternalOutput")
        num_dbg = nc.dram_tensor("num_dbg", [P, TPB], F32,
                                 kind="ExternalOutput")
        io_dbg = nc.dram_tensor("io_dbg", [P, P], F32,
                                kind="ExternalOutput")
        ms_dbg = nc.dram_tensor("ms_dbg", [P, TPB * P], F32,
                                kind="ExternalOutput")

    groups = [list(range(NCORES))]

    with tile.TileContext(nc) as tc:
        with tc.tile_pool(name="const", bufs=1) as cst:
            fcwT_sb = cst.tile([P, P], F32)
            nc.sync.dma_start(fcwT_sb[:], fc_wT[:])
            w12_sb = cst.tile([P, 2], F32)
            nc.sync.dma_start(w12_sb[:], w12[:])
            pcr_sb = cst.tile([P, P], F32)
            nc.sync.dma_start(pcr_sb[:], pc_rep[:])
            ipc_sb = cst.tile([P, 1], F32)
            nc.sync.dma_start(ipc_sb[:], ipc_col[:])
            gb_sb = cst.tile([P, 2], F32)
            nc.sync.dma_start(gb_sb[:], gb2[:])
            npad_sb = cst.tile([P, 1], F32)
            nc.sync.dma_start(npad_sb[:], negpad[:])
            iota_sb = cst.tile([P, P], F32)
            nc.gpsimd.iota(iota_sb[:], pattern=[[1, P]], channel_multiplier=0,
                           allow_small_or_imprecise_dtypes=True)
            ident_sb = cst.tile([P, P], F32)
            make_identity(nc, ident_sb[:])
            # ln(EPS) column for the pad-count stats correction
            lneps_sb = cst.tile([P, 1], F32)
            nc.gpsimd.memset(lneps_sb[:], float(math.log(EPS)))
            zb = cst.tile([P, 1], F32)
            nc.gpsimd.memset(zb[:], 0.0)
            epsb = cst.tile([P, 1], F32)
            nc.gpsimd.memset(epsb[:], EPS)
            bnepsb = cst.tile([P, 1], F32)
            nc.gpsimd.memset(bnepsb[:], BN_EPS)
            # zero-fill lnden_d so indirect reads never see uninit DRAM
            lz0 = cst.tile([P, B], F32)
            nc.gpsimd.memset(lz0[:], 0.0)
            nc.gpsimd.dma_start(
                lnden_d[:].rearrange("(p x) o -> p (x o)", p=P), lz0[:])

            # persistent stores
            s_store = cst.tile([P, B * P], F32)
            hn_store = cst.tile([P, B * P], F32)
            sumh = cst.tile([P, B], F32)
            sumq = cst.tile([P, B], F32)

            # ================= phase 0: z / GP / s1 / s2 tables =============
            with (
                tc.tile_pool(name="p0", bufs=3) as p0,
                tc.tile_pool(name="p0ps", bufs=2, space="PSUM") as p0ps,
            ):
                for i in range(T0):
                    ht = p0.tile([P, P], F32, tag="ht")
                    nc.sync.dma_start(ht[:], hT_l[:, i * P:(i + 1) * P])
                    zps = p0ps.tile([P, P], F32, tag="zps")
                    nc.tensor.matmul(zps[:], ht[:], fcwT_sb[:],
                                     start=True, stop=True)
                    s12 = p0ps.tile([P, 2], F32, tag="s12")
                    nc.tensor.matmul(s12[:], ht[:], w12_sb[:],
                                     start=True, stop=True)
                    az = p0.tile([P, P], F32, tag="az")
                    nc.scalar.activation(az[:], zps[:], AF.Abs, bias=zb[:])
                    lnz = p0.tile([P, P], F32, tag="lnz")
                    nc.scalar.activation(lnz[:], az[:], AF.Ln, bias=zb[:])
                    zx = p0.tile([P, ZC], F32, tag="zx")
                    nc.vector.tensor_tensor(zx[:, 0:P], lnz[:], pcr_sb[:],
                                            op=ALU.mult)
                    nc.vector.tensor_copy(zx[:, P:P + 1], s12[:, 0:1])
                    nc.gpsimd.memset(zx[:, P + 1:ZC], 0.0)
                    s2c = p0.tile([P, 1], F32, tag="s2c")
                    nc.vector.tensor_copy(s2c[:], s12[:, 1:2])
                    nc.sync.dma_start(zext_l[i * P:(i + 1) * P, :], zx[:])
                    nc.sync.dma_start(s2_l[i * P:(i + 1) * P, :], s2c[:])

            # ================= allgather tables =============================
            nc.gpsimd.collective_compute(
                "AllGather", ALU.bypass, replica_groups=groups,
                ins=[zext_l[:]], outs=[z_full[:]])
            nc.gpsimd.collective_compute(
                "AllGather", ALU.bypass, replica_groups=groups,
                ins=[s2_l[:]], outs=[s2_full[:]])

            # ================= edge phase ===================================
            with (
                tc.tile_pool(name="eidx", bufs=3) as eidx,
                tc.tile_pool(name="egg", bufs=3) as egg,
                tc.tile_pool(name="esm", bufs=4) as esm,
                tc.tile_pool(name="ework", bufs=4) as ework,
                tc.tile_pool(name="eps1", bufs=2, space="PSUM") as epsA,
                tc.tile_pool(name="eps2", bufs=3, space="PSUM") as epsB,
            ):
                for b in range(B):
                    srcb = eidx.tile([P, TPB], I32, tag="srcb")
                    nc.sync.dma_start(srcb[:], srcg[b][:])
                    dmb = eidx.tile([P, TPB], F32, tag="dmb")
                    nc.sync.dma_start(dmb[:], dstmf[b][:])
                    dgb = eidx.tile([P, TPB], I32, tag="dgb")
                    nc.sync.dma_start(dgb[:], dstg[b][:])
                    dlb = eidx.tile([P, TPB], I32, tag="dlb")
                    nc.sync.dma_start(dlb[:], dstl[b][:])
                    pnb = eidx.tile([P, TPB], F32, tag="pnb")
                    nc.sync.dma_start(pnb[:], pneg[b][:])

                    gg = egg.tile([P, TPB, ZC], F32, tag="gg")
                    for t in range(TPB):
                        nc.gpsimd.indirect_dma_start(
                            out=gg[:, t, :], out_offset=None, in_=z_full[:],
                            in_offset=bass.IndirectOffsetOnAxis(
                                ap=srcb[:, t:t + 1], axis=0))
                    s2v = esm.tile([P, TPB, 1], F32, tag="s2v")
                    for t in range(TPB):
                        nc.gpsimd.indirect_dma_start(
                            out=s2v[:, t, :], out_offset=None, in_=s2_full[:],
                            in_offset=bass.IndirectOffsetOnAxis(
                                ap=dgb[:, t:t + 1], axis=0))

                    # e = lrelu(s1[src] + s2[dst]) with -1e30 on pad slots
                    ev = esm.tile([P, TPB], F32, tag="ev")
                    nc.vector.tensor_tensor(ev[:], gg[:, :, P], s2v[:, :, 0],
                                            op=ALU.add)
                    nc.vector.tensor_tensor(ev[:], ev[:], pnb[:], op=ALU.add)
                    elr = esm.tile([P, TPB], F32, tag="elr")
                    nc.vector.scalar_tensor_tensor(
                        elr[:], in0=ev[:], scalar=0.01, in1=ev[:],
                        op0=ALU.mult, op1=ALU.max)
                    num = esm.tile([P, TPB], F32, tag="num")
                    nc.scalar.activation(num[:], elr[:], AF.Exp, bias=zb[:])
                    numb = esm.tile([P, TPB], F16, tag="numb")
                    nc.vector.tensor_copy(numb[:], num[:])

                    # selection matrix M[e, t, j] = (dstm[e,t] == j)
                    msel = egg.tile([P, TPB, P], F16, tag="msel")
                    nc.vector.tensor_tensor(
                        msel[:],
                        dmb[:, :, None].broadcast_to([P, TPB, P]),
                        iota_sb[:, None, :].broadcast_to([P, TPB, P]),
                        op=ALU.is_equal)

                    if debug_outs and b == 0:
                        nc.sync.dma_start(
                            gg_dbg[:], gg[:].rearrange("p t c -> p (t c)"))
                        nc.sync.dma_start(s2v_dbg[:], s2v[:, :, 0])
                        nc.sync.dma_start(num_dbg[:], num[:])
                        nc.sync.dma_start(io_dbg[:], iota_sb[:])
                        msf = ework.tile([P, TPB * P], F32, tag="msf")
                        nc.vector.tensor_copy(
                            msf[:], msel[:].rearrange("p t c -> p (t c)"))
                        nc.sync.dma_start(ms_dbg[:], msf[:])
                    denp = epsA.tile([P, 1], F32, tag="denp")
                    for t in range(TPB):
                        nc.tensor.matmul(denp[:], msel[:, t, :],
                                         numb[:, t:t + 1],
                                         start=(t == 0), stop=(t == TPB - 1))
                    dclamp = esm.tile([P, 1], F32, tag="dclamp")
                    nc.vector.tensor_scalar(dclamp[:], denp[:], 1e-30, None,
                                            ALU.max)
                    lnden = esm.tile([P, 1], F32, tag="lnden")
                    nc.scalar.activation(lnden[:], dclamp[:], AF.Ln, bias=zb[:])
                    nc.sync.dma_start(lnden_d[b * P:(b + 1) * P, :], lnden[:])
                    lndv = esm.tile([P, TPB, 1], F32, tag="lndv")
                    for t in range(TPB):
                        nc.gpsimd.indirect_dma_start(
                            out=lndv[:, t, :], out_offset=None, in_=lnden_d[:],
                            in_offset=bass.IndirectOffsetOnAxis(
                                ap=dlb[:, t:t + 1], axis=0))
                    lnnum = esm.tile([P, TPB], F32, tag="lnnum")
                    nc.scalar.activation(lnnum[:], numb[:], AF.Ln, bias=zb[:])
                    lna = esm.tile([P, TPB], F32, tag="lna")
                    nc.vector.tensor_tensor(lna[:], lnnum[:], lndv[:, :, 0],
                                            op=ALU.subtract)

                    sacc = epsB.tile([P, P], F32, tag="sacc")
                    for t in range(TPB):
                        pre = ework.tile([P, P], F32, tag="pre")
                        nc.vector.scalar_tensor_tensor(
                            pre[:], in0=pcr_sb[:], scalar=lna[:, t:t + 1],
                            in1=gg[:, t, 0:P], op0=ALU.mult, op1=ALU.add)
                        ctrb = ework.tile([P, P], F16, tag="ctrb")
                        nc.scalar.activation(ctrb[:], pre[:], AF.Exp, bias=zb[:])
                        nc.tensor.matmul(sacc[:], ctrb[:], msel[:, t, :],
                                         start=(t == 0), stop=(t == TPB - 1))
                    nc.scalar.activation(s_store[:, b * P:(b + 1) * P],
                                         sacc[:], AF.Copy)

            if debug_outs:
                nc.sync.dma_start(zf_dbg[:], z_full[:])
                nc.sync.dma_start(s2_dbg[:], s2_full[:])

            # ================= post: hn, BN stats ===========================
            with (
                tc.tile_pool(name="post", bufs=3) as po,
                tc.tile_pool(name="pops", bufs=2, space="PSUM") as pops,
            ):
                for b in range(B):
                    sb = s_store[:, b * P:(b + 1) * P]
                    lnb = po.tile([P, P], F32, tag="lnb")
                    nc.scalar.activation(lnb[:], sb, AF.Ln, bias=epsb[:])
                    nc.scalar.activation(hn_store[:, b * P:(b + 1) * P],
                                         lnb[:], AF.Exp, bias=zb[:],
                                         scale=ipc_sb[:],
                                         accum_out=sumh[:, b:b + 1])
                    hnq = po.tile([P, P], F32, tag="hnq")
                    nc.scalar.activation(hnq[:],
                                         hn_store[:, b * P:(b + 1) * P],
                                         AF.Square, bias=zb[:],
                                         accum_out=sumq[:, b:b + 1])

                # raw per-core sums, pad-corrected:  sum -= npad * eps^(1/pc)
                epspow = po.tile([P, 1], F32)
                nc.scalar.activation(epspow[:], lneps_sb[:], AF.Exp,
                                     bias=zb[:], scale=ipc_sb[:])
                epsq = po.tile([P, 1], F32)
                nc.scalar.activation(epsq[:], epspow[:], AF.Square, bias=zb[:])
                rs_h = po.tile([P, 1], F32)
                nc.vector.tensor_reduce(rs_h[:], sumh[:], mybir.AxisListType.X,
                                        ALU.add)
                rs_q = po.tile([P, 1], F32)
                nc.vector.tensor_reduce(rs_q[:], sumq[:], mybir.AxisListType.X,
                                        ALU.add)
                stat = po.tile([P, 2], F32)
                nc.vector.scalar_tensor_tensor(
                    stat[:, 0:1], in0=epspow[:], scalar=npad_sb[:],
                    in1=rs_h[:], op0=ALU.mult, op1=ALU.add)
                nc.vector.scalar_tensor_tensor(
                    stat[:, 1:2], in0=epsq[:], scalar=npad_sb[:],
                    in1=rs_q[:], op0=ALU.mult, op1=ALU.add)
                nc.sync.dma_start(st_in[:], stat[:])
                nc.gpsimd.collective_compute(
                    "AllReduce", ALU.add, replica_groups=groups,
                    ins=[st_in[:]], outs=[st_out[:]])
                gstat = po.tile([P, 2], F32)
                nc.sync.dma_start(gstat[:], st_out[:])

                invn = 1.0 / float(N_total)
                mean = po.tile([P, 1], F32)
                nc.vector.tensor_scalar(mean[:], gstat[:, 0:1], invn, None,
                                        ALU.mult)
                exq = po.tile([P, 1], F32)
                nc.vector.tensor_scalar(exq[:], gstat[:, 1:2], invn, None,
                                        ALU.mult)
                nmean = po.tile([P, 1], F32)
                nc.vector.tensor_scalar(nmean[:], mean[:], -1.0, None,
                                        ALU.mult)
                var = po.tile([P, 1], F32)
                nc.vector.scalar_tensor_tensor(
                    var[:], in0=mean[:], scalar=nmean[:], in1=exq[:],
                    op0=ALU.mult, op1=ALU.add)
                # rstd = exp(-0.5 * ln(var + bn_eps))  (stays in the act table)
                lv = po.tile([P, 1], F32)
                nc.scalar.activation(lv[:], var[:], AF.Ln, bias=bnepsb[:])
                rstd = po.tile([P, 1], F32)
                nc.scalar.activation(rstd[:], lv[:], AF.Exp, bias=zb[:], scale=-0.5)
                acol = po.tile([P, 1], F32)
                nc.vector.tensor_tensor(acol[:], gb_sb[:, 0:1], rstd[:],
                                        op=ALU.mult)
                nacol = po.tile([P, 1], F32)
                nc.vector.tensor_scalar(nacol[:], acol[:], -1.0, None,
                                        ALU.mult)
                bcol = po.tile([P, 1], F32)
                nc.vector.scalar_tensor_tensor(
                    bcol[:], in0=mean[:], scalar=nacol[:], in1=gb_sb[:, 1:2],
                    op0=ALU.mult, op1=ALU.add)

                # normalize + elu + transpose + write out
                for b in range(B):
                    v = po.tile([P, P], F32, tag="v")
                    nc.vector.tensor_scalar(v[:],
                                            hn_store[:, b * P:(b + 1) * P],
                                            acol[:], bcol[:],
                                            ALU.mult, ALU.add)
                    em = po.tile([P, P], F32, tag="em")
                    nc.scalar.activation(em[:], v[:], AF.Exp, bias=zb[:])
                    t1 = po.tile([P, P], F32, tag="t1")
                    nc.vector.tensor_scalar(t1[:], em[:], 1.0, 0.0,
                                            ALU.subtract, ALU.min)
                    el = po.tile([P, P], F32, tag="el")
                    nc.vector.scalar_tensor_tensor(
                        el[:], in0=v[:], scalar=0.0, in1=t1[:],
                        op0=ALU.max, op1=ALU.add)
                    tp = pops.tile([P, P], F32, tag="tp")
                    nc.tensor.transpose(tp[:], el[:], ident_sb[:])
                    ot = po.tile([P, P], F32, tag="ot")
                    nc.scalar.activation(ot[:], tp[:], AF.Copy)
                    nc.sync.dma_start(out_d[b * P:(b + 1) * P, :], ot[:])
                if debug_outs:
                    nc.sync.dma_start(ld_dbg[:], lnden_d[:])
                    nc.sync.dma_start(st_dbg[:], st_out[:])

    nc.compile()
    return nc


def _host_prep(h, src, dst, fc_w, attn_w, p, bn_gamma, bn_beta):
    N, DIN = h.shape
    DOUT = fc_w.shape[0]
    assert DIN == P and DOUT == P
    E = src.shape[0]

    # phase-0 split
    T0 = int(math.ceil(N / (NCORES * P)))
    ZROWS = NCORES * T0 * P
    hT = np.ones((P, ZROWS), dtype=np.float32)
    hT[:, :N] = np.ascontiguousarray(h.T)

    # sort edges by dst
    order = np.argsort(dst, kind="stable")
    src_s = np.ascontiguousarray(src[order]).astype(np.int64)
    dst_s = np.ascontiguousarray(dst[order]).astype(np.int64)
    deg = np.bincount(dst_s, minlength=N)
    starts = np.zeros(N + 1, dtype=np.int64)
    np.cumsum(deg, out=starts[1:])
    assert deg.max() <= BE, "single node exceeds block capacity"

    # greedy blocks: <=P nodes, <=BE edges, whole nodes
    blk_lo, blk_hi = [], []          # node ranges
    n = 0
    while n < N:
        m = int(np.searchsorted(starts, starts[n] + BE, side="right")) - 1
        m = min(m, n + P, N)
        m = max(m, n + 1)
        blk_lo.append(n)
        blk_hi.append(m)
        n = m
    nb = len(blk_lo)
    B = int(math.ceil(nb / NCORES))
    blk_lo = np.array(blk_lo + [N] * (NCORES * B - nb), dtype=np.int64)
    blk_hi = np.array(blk_hi + [N] * (NCORES * B - nb), dtype=np.int64)

    # per-core per-block slot arrays
    srcg = np.zeros((NCORES, B, P, TPB), dtype=np.int32)
    dstmf = np.zeros((NCORES, B, P, TPB), dtype=np.float32)
    dstg = np.zeros((NCORES, B, P, TPB), dtype=np.int32)
    dstl = np.zeros((NCORES, B, P, TPB), dtype=np.int32)
    pneg = np.full((NCORES, B, P, TPB), NEG_BIG, dtype=np.float32)
    for c in range(NCORES):
        for b in range(B):
            g = c * B + b
            lo, hi = blk_lo[g], blk_hi[g]
            e0, e1 = starts[lo], starts[hi]
            k = int(e1 - e0)
            dstl[c, b] += b * P  # pad slots still point at a valid row
            if k == 0:
                continue
            off = np.arange(k)
            pp = off % P
            tt = off // P
            srcg[c, b, pp, tt] = src_s[e0:e1]
            dm = (dst_s[e0:e1] - lo)
            dstmf[c, b, pp, tt] = dm.astype(np.float32)
            dstg[c, b, pp, tt] = dst_s[e0:e1]
            dstl[c, b, pp, tt] = (b * P + dm).astype(np.int32)
            pneg[c, b, pp, tt] = 0.0

    # weights / constants
    a_src = attn_w[0, :DOUT].astype(np.float32)
    a_dst = attn_w[0, DOUT:].astype(np.float32)
    fc_wT = np.ascontiguousarray(fc_w.T.astype(np.float32))
    w12 = np.ascontiguousarray(fc_wT @ np.stack([a_src, a_dst], 1))
    pc = np.clip(p.astype(np.float32), 1.0, 100.0)
    pc_rep = np.ascontiguousarray(np.tile(pc[None, :], (P, 1)))
    ipc_col = np.ascontiguousarray((1.0 / pc)[:, None])
    gb2 = np.ascontiguousarray(
        np.stack([bn_gamma.astype(np.float32), bn_beta.astype(np.float32)], 1))

    total_slots = NCORES * B * P
    npad_global = float(total_slots - N)

    in_maps = []
    for c in range(NCORES):
        # per-core pad slots do not matter here: correction uses the global
        # count, applied identically on every core before the AllReduce —
        # sum over cores counts each core's own pads once.
        nslots_c = B * P
        nreal_c = int(np.sum(np.minimum(blk_hi[c * B:(c + 1) * B], N)
                             - np.minimum(blk_lo[c * B:(c + 1) * B], N)))
        npad_c = float(nslots_c - nreal_c)
        in_maps.append({
            "hT_l": np.ascontiguousarray(
                hT[:, c * T0 * P:(c + 1) * T0 * P]),
            "fc_wT": fc_wT,
            "w12": w12.astype(np.float32),
            "pc_rep": pc_rep,
            "ipc_col": ipc_col,
            "gb2": gb2,
            "negpad": np.full((P, 1), -npad_c, dtype=np.float32),
            "srcg": srcg[c],
            "dstmf": dstmf[c],
            "dstg": dstg[c],
            "dstl": dstl[c],
            "pneg": pneg[c],
        })

    meta = dict(B=B, T0=T0, N=N, blk_lo=blk_lo, blk_hi=blk_hi)
    return in_maps, meta


def kernel(h, src, dst, fc_w, attn_w, p, bn_gamma, bn_beta):
    in_maps, meta = _host_prep(h, src, dst, fc_w, attn_w, p,
                               bn_gamma, bn_beta)
    B, T0, N = meta["B"], meta["T0"], meta["N"]
    nc = _build_nc(B, T0, N)

    from concourse.bass_utils import run_bass_kernel_spmd
    res = run_bass_kernel_spmd(nc, in_maps, core_ids=list(range(NCORES)))
    global _last_results, _last_ctx
    _last_results = res
    _last_ctx = (nc, in_maps)
    outs = [r["out"] for r in res.results]

    blk_lo, blk_hi = meta["blk_lo"], meta["blk_hi"]
    full = np.empty((N, P), dtype=np.float32)
    for c in range(NCORES):
        for b in range(B):
            g = c * B + b
            lo, hi = int(blk_lo[g]), int(blk_hi[g])
            cnt = hi - lo
            if cnt > 0:
                full[lo:hi] = outs[c][b * P:b * P + cnt]
    return full


def bench_warm(n=3):
    """Re-execute the last-built program n times (jit cache warm); min wall."""
    import time
    from concourse.bass_utils import run_bass_kernel_spmd
    nc, in_maps = _last_ctx
    best = float("inf")
    for _ in range(n):
        t0 = time.time()
        run_bass_kernel_spmd(nc, in_maps, core_ids=list(range(NCORES)))
        best = min(best, time.time() - t0)
    return best * 1e9
